# revision 28
# baseline (speedup 1.0000x reference)
"""Trainium2 Bass kernel: 2-layer LSTM decoder (B=512 T=128 E=64 H=512 V=100).

Strategy: pure data parallelism, batch 512 -> 8 cores x 64.
Per core, everything is fp32 and all weights stay resident in SBUF.

Matmul mapping ("x-stationary"): out = gates[batch, feат] in PSUM,
stationary lhsT = transposed activations [K,64], moving rhs = weights
[K, cols]. The 128-wide PE array is split into two 64-col groups
(tile_position) so both batch-halves of the gate features compute
concurrently: PSUM gate tiles are [128, 512] = [batch(64) x 2 feature
halves, i|f or j|o gate pair], host pre-permutes weight columns into
[i_lo f_lo j_lo o_lo | i_hi f_hi j_hi o_hi] blocks of 256.

Cell elementwise runs on [128, 256] tiles (feature halves stacked on
partitions). dynamic_rnn length masking is exact and branch-free:
  - layer0: a rider row on the embedding one-hot matmul adds
    -1e9*(t>=len) to i-gate pre-activations and +1e9*(t>=len) to f-gate
    (c then freezes exactly; h past the length is garbage but provably
    never reaches the output).
  - layer1: same clamp via per-partition ACT bias columns.
  - output: scores_t = (mask * h1_t) @ Wp [+ bp].
Embedding lookup = one-hot matmul against a precomputed E @ W0_emb
table; the state (t-invariant) contribution + b0 + forget_bias is
precomputed once and re-added to PSUM each step via an identity matmul.
"""

import sys
import types

import ml_dtypes
import numpy as np

import concourse.bass as bass
import concourse.mybir as mybir
import concourse.tile as tile
from concourse.bass_utils import run_bass_kernel_spmd
from concourse.masks import make_identity

B, T, E, H, V = 512, 128, 64, 512, 100
NCORES = 8
BL = B // NCORES          # 64 rows of batch per core
G = 4 * H                 # 2048 gate columns
F32 = mybir.dt.float32
F32R = mybir.dt.float32r
BF16 = mybir.dt.bfloat16
AF = mybir.ActivationFunctionType
ALU = mybir.AluOpType
CLAMP = 1.0e9


def _perm_cols():
    # reference gate order is [i j f o]*512; regroup to
    # [i_lo f_lo j_lo o_lo | i_hi f_hi j_hi o_hi] blocks of 256
    idx = []
    for g in (0, 1):
        lo = 256 * g
        for base in (0, 1024, 512, 1536):  # i, f, j, o
            idx.extend(range(base + lo, base + lo + 256))
    return np.asarray(idx)


def _hT(tile_, k):
    # transposed-activation K-chunk k (feats 128k:128k+128) as [128, 64];
    # tile_ is [128, 256] = [block0 (feats 0:128,256:384) | block1]
    off = 128 * (k % 2) + 64 * (k // 2)
    return tile_[:, off:off + 64]


def _build_module(with_b1, with_bp):
    nc = bass.Bass(target_bir_lowering=False)

    def din(name, shape, dt=F32):
        return nc.dram_tensor(name, list(shape), dt, kind="ExternalInput")

    tok_bc = din("tok_bc", (V, T * BL))       # token id per (t,b) col, replicated
    len_bc = din("len_bc", (128, BL))         # lengths replicated over partitions
    len_col = din("len_col", (128, 1))        # lengths duplicated per partition half
    iota_v = din("iota_v", (V, 1))            # 0..99
    iota_t = din("iota_t", (128, 1))          # 0..127
    iota_bc = din("iota_bc", (128, 128))      # 0..127 per col, replicated
    sign_ce = din("sign_ce", (1, G), BF16)          # -+1e9 on i/f cols, 0 on j/o
    ones2 = din("ones2", (2, BL), F32R)
    stateT = din("stateT", (H, BL), F32R)
    et = din("et", (E, V), F32R)                    # char_embeddings.T
    w0e = din("w0e", (E, G), F32R)                  # permuted column blocks
    w0s = din("w0s", (H, G), F32R)
    w0h = din("w0h", (H, G), BF16)
    w1 = din("w1", (2 * H, G), BF16)
    wp = din("wp", (H, V), F32R)
    b0f = din("b0f", (2, G), F32R)                  # [perm(b0); +1 on f cols]
    b1p = din("b1p", (1, G), F32R) if with_b1 else None
    ones1 = din("ones1", (1, BL), F32R) if (with_bp or with_b1) else None
    bpp = din("bp", (1, V), F32R) if with_bp else None

    scores = nc.dram_tensor("scores", [T, BL, V], F32, kind="ExternalOutput")

    with tile.TileContext(nc) as tc:
        with tc.tile_pool(name="persist", bufs=1) as pp:
            # ---------------- persistent SBUF ----------------
            w0h_sb = [pp.tile([128, G], BF16, name=f"w0h{k}", tag=f"w0h{k}") for k in range(4)]
            w1_sb = [pp.tile([128, G], BF16, name=f"w1{k}", tag=f"w1{k}") for k in range(8)]
            wp_sb = [pp.tile([128, V], F32R, name=f"wp{k}", tag=f"wp{k}") for k in range(4)]
            wp16_sb = [pp.tile([128, V], BF16, name=f"wpb{k}", tag=f"wpb{k}") for k in range(4)]
            oh = pp.tile([V + 1, T * BL], BF16, name="oh", tag="oh")
            ce = pp.tile([V + 1, G], BF16, name="ce", tag="ce")
            sc_if = pp.tile([128, 512], F32R, name="sc_if", tag="sc_if")
            sc_jo = pp.tile([128, 512], F32R, name="sc_jo", tag="sc_jo")
            identf = pp.tile([128, 128], F32, name="identf", tag="identf")
            ident = pp.tile([128, 128], F32R, name="ident", tag="ident")
            identb = pp.tile([128, 128], BF16, name="identb", tag="identb")
            maskcols = pp.tile([128, 128], F32, name="maskcols", tag="maskcols")
            iclamp = pp.tile([128, 128], F32, name="iclamp", tag="iclamp")
            fclamp = pp.tile([128, 128], F32, name="fclamp", tag="fclamp")
            invm = pp.tile([128, BL], BF16, name="invm", tag="invm")
            ones2_sb = pp.tile([2, BL], F32R, name="ones2", tag="ones2")
            c0 = pp.tile([128, 256], F32, name="c0", tag="c0")
            c1 = pp.tile([128, 256], F32, name="c1", tag="c1")
            hz = [pp.tile([128, 256], BF16, name=f"hz{i}", tag=f"hz{i}") for i in range(2)]
            if with_b1:
                b1_sb = pp.tile([1, G], F32R, name="b1", tag="b1")
            if with_b1 or with_bp:
                ones1_sb = pp.tile([1, BL], F32R, name="ones1", tag="ones1")
            if with_bp:
                bp_sb = pp.tile([1, V], F32R, name="bp", tag="bp")

            for k in range(4):
                nc.sync.dma_start(w0h_sb[k][:], w0h[128 * k:128 * k + 128, :])
            for k in range(4):
                nc.sync.dma_start(w1_sb[k][:], w1[128 * k:128 * k + 128, :])
                nc.sync.dma_start(w1_sb[4 + k][:], w1[512 + 128 * k:512 + 128 * k + 128, :])
            for k in range(4):
                nc.sync.dma_start(wp_sb[k][:], wp[128 * k:128 * k + 128, :])
                nc.vector.tensor_copy(wp16_sb[k][:], wp_sb[k][:])
            nc.sync.dma_start(ones2_sb[:], ones2[:])
            if with_b1:
                nc.sync.dma_start(b1_sb[:], b1p[:])
            if with_b1 or with_bp:
                nc.sync.dma_start(ones1_sb[:], ones1[:])
            if with_bp:
                nc.sync.dma_start(bp_sb[:], bpp[:])

            make_identity(nc, identf[:])
            nc.vector.tensor_copy(ident[:], identf[:])
            nc.vector.tensor_copy(identb[:], identf[:])
            nc.gpsimd.memset(c0[:], 0.0)
            nc.gpsimd.memset(c1[:], 0.0)
            zf = pp.tile([128, 256], F32, name="zf", tag="zf")
            nc.gpsimd.memset(zf[:], 0.0)
            for z in hz:
                nc.vector.tensor_copy(z[:], zf[:])

            # ---------------- startup compute ----------------
            with (
                tc.tile_pool(name="boot", bufs=2) as bp_pool,
                tc.tile_pool(name="boot1", bufs=1) as b1_pool,
                tc.tile_pool(name="psum_boot", bufs=1, space="PSUM") as pb,
            ):
                # masks
                lb = b1_pool.tile([128, BL], F32, name="len_bc", tag="len_bc")
                nc.sync.dma_start(lb[:], len_bc[:])
                lc = b1_pool.tile([128, 1], F32, name="len_col", tag="len_col")
                nc.sync.dma_start(lc[:], len_col[:])
                it_ = b1_pool.tile([128, 1], F32, name="iota_t", tag="iota_t")
                nc.sync.dma_start(it_[:], iota_t[:])
                ib = b1_pool.tile([128, 128], F32, name="iota_bc", tag="iota_bc")
                nc.sync.dma_start(ib[:], iota_bc[:])
                iv = b1_pool.tile([V, 1], F32, name="iota_v", tag="iota_v")
                nc.sync.dma_start(iv[:], iota_v[:])

                # invm[t,b] = (len[b] <= t)
                nc.vector.tensor_scalar(invm[:], lb[:], it_[:], None, ALU.is_le)
                # maskcols[p,t] = (t < len[p%64])
                nc.vector.tensor_scalar(maskcols[:], ib[:], lc[:], None, ALU.is_lt)
                # iclamp = (m-1)*1e9 ; fclamp = (1-m)*1e9 + 1
                nc.vector.tensor_scalar(iclamp[:], maskcols[:], 1.0, CLAMP,
                                        ALU.subtract, ALU.mult)
                nc.vector.tensor_scalar(fclamp[:], maskcols[:], 1.0, -CLAMP,
                                        ALU.subtract, ALU.mult)
                nc.vector.tensor_scalar_add(fclamp[:], fclamp[:], 1.0)

                # one-hot token table, built in column chunks
                CH = 1024
                for cidx in range(T * BL // CH):
                    tb = bp_pool.tile([V, CH], F32, name="tok_bc", tag="tok_bc")
                    nc.sync.dma_start(tb[:], tok_bc[:, CH * cidx:CH * (cidx + 1)])
                    nc.vector.tensor_scalar(oh[0:V, CH * cidx:CH * (cidx + 1)],
                                            tb[:], iv[:], None, ALU.is_equal)
                # rider row: invm flattened t-major
                nc.sync.dma_start(oh[V:V + 1, :], invm[:])

                # ce = [E @ w0e ; sign row]
                et_sb = b1_pool.tile([E, V], F32R, name="et", tag="et")
                nc.sync.dma_start(et_sb[:], et[:])
                w0e_sb = b1_pool.tile([E, G], F32R, name="w0e", tag="w0e")
                nc.sync.dma_start(w0e_sb[:], w0e[:])
                for n in range(4):
                    cps = pb.tile([V, 512], F32, name="ce_ps", tag="ce_ps")
                    nc.tensor.matmul(cps[:], et_sb[:], w0e_sb[:, 512 * n:512 * n + 512],
                                     start=True, stop=True)
                    nc.scalar.copy(ce[0:V, 512 * n:512 * n + 512], cps[:])
                nc.sync.dma_start(ce[V:V + 1, :], sign_ce[:])

                # sc = state @ w0s + b0 + forget_bias, in gate-tile layout
                st_sb = [b1_pool.tile([128, BL], F32R, name=f"st{k}", tag=f"st{k}") for k in range(4)]
                for k in range(4):
                    nc.sync.dma_start(st_sb[k][:], stateT[128 * k:128 * k + 128, :])
                b0f_sb = b1_pool.tile([2, G], F32R, name="b0f", tag="b0f")
                nc.sync.dma_start(b0f_sb[:], b0f[:])
                for h2 in (0, 1):
                    cb = 1024 * h2
                    ob = 64 * h2
                    ps_if = pb.tile([64, 512], F32, name="sc_ps_if", tag="sc_ps_if")
                    ps_jo = pb.tile([64, 512], F32, name="sc_ps_jo", tag="sc_ps_jo")
                    for k in range(4):
                        w0s_k = bp_pool.tile([128, G], F32R, name="w0s", tag="w0s")
                        nc.sync.dma_start(w0s_k[:], w0s[128 * k:128 * k + 128, :])
                        nc.tensor.matmul(ps_if[:], st_sb[k][:], w0s_k[:, cb:cb + 512],
                                         start=(k == 0), stop=False)
                        nc.tensor.matmul(ps_jo[:], st_sb[k][:],
                                         w0s_k[:, cb + 512:cb + 1024],
                                         start=(k == 0), stop=False)
                    nc.tensor.matmul(ps_if[:], ones2_sb[:], b0f_sb[:, cb:cb + 512],
                                     start=False, stop=True)
                    nc.tensor.matmul(ps_jo[:], ones2_sb[:],
                                     b0f_sb[:, cb + 512:cb + 1024],
                                     start=False, stop=True)
                    nc.scalar.copy(sc_if[64 * h2:64 * h2 + 64, :], ps_if[:])
                    nc.scalar.copy(sc_jo[64 * h2:64 * h2 + 64, :], ps_jo[:])

            # ---------------- recurrence ----------------
            with (
                tc.tile_pool(name="act", bufs=4) as pa,
                tc.tile_pool(name="cell", bufs=4) as pc,
                tc.tile_pool(name="ht", bufs=4) as ph,
                tc.tile_pool(name="psum_g", bufs=1, space="PSUM") as pg,
                tc.tile_pool(name="psum_tp", bufs=1, space="PSUM") as ptp,
                tc.tile_pool(name="psum_s", bufs=1, space="PSUM") as psp,
            ):
                h0T = hz[0]
                h1T = hz[1]
                pending = None  # (t, h1n[, tc1, so1]) deferred tail

                def emit_tail(pt, h1n_p, tc1_p, so1_p):
                    # h1 transpose for the recurrence
                    tp1 = ptp.tile([128, 256], BF16, name="tp1", tag="tp")
                    for blk in (0, 1):
                        sl = slice(128 * blk, 128 * blk + 128)
                        nc.tensor.transpose(tp1[:, sl], h1n_p[:, sl], identb[:])
                    hT = ph.tile([128, 256], BF16, name="h1T", tag="h1T")
                    nc.vector.tensor_copy(hT[:], tp1[:])
                    # masked projection
                    s = psp.tile([BL, V], F32, name="s", tag="s")
                    if with_bp:
                        h1m = pc.tile([128, 256], F32R, name="h1m", tag="h1m")
                        nc.vector.scalar_tensor_tensor(
                            h1m[:], tc1_p[:], maskcols[:, pt:pt + 1], so1_p[:],
                            ALU.mult, ALU.mult)
                        tpm = ptp.tile([128, 256], F32R, name="tpm", tag="tpr",
                                       bufs=1)
                        for blk in (0, 1):
                            nc.tensor.transpose(
                                tpm[:, 128 * blk:128 * blk + 128],
                                h1m[:, 128 * blk:128 * blk + 128], ident[:])
                        hmT = ph.tile([128, 256], F32R, name="hmT", tag="hmT")
                        nc.vector.tensor_copy(hmT[:], tpm[:])
                        for k in range(4):
                            nc.tensor.matmul(s[:], _hT(hmT, k), wp_sb[k][:],
                                             start=(k == 0), stop=False)
                        nc.tensor.matmul(s[:], ones1_sb[:], bp_sb[:],
                                         start=False, stop=True)
                        s_sb = pc.tile([BL, V], F32, name="s_sb", tag="s_sb")
                        nc.scalar.copy(s_sb[:], s[:])
                    else:
                        for k in range(4):
                            nc.tensor.matmul(s[:], _hT(hT, k), wp16_sb[k][:],
                                             start=(k == 0), stop=(k == 3))
                        s_sb = pc.tile([BL, V], F32, name="s_sb", tag="s_sb")
                        nc.vector.tensor_scalar_mul(s_sb[:], s[:],
                                                    maskcols[0:BL, pt:pt + 1])
                    nc.sync.dma_start(scores[pt], s_sb[:])
                    return hT

                for t in range(T):
                    ohs = oh[:, BL * t:BL * (t + 1)]

                    # ----- layer0 gate accumulation -----
                    g0_if = pg.tile([128, 512], F32, name="g0if", tag="g0if")
                    g0_jo = pg.tile([128, 512], F32, name="g0jo", tag="g0jo")
                    for gt, gtile in ((0, g0_if), (512, g0_jo)):
                        for h2 in (0, 1):
                            nc.tensor.matmul(
                                gtile[64 * h2:64 * h2 + 64, :], ohs,
                                ce[:, 1024 * h2 + gt:1024 * h2 + gt + 512],
                                start=True, stop=False)
                    # state contribution added on DVE, early (before the
                    # h-part matmuls accumulate on top; PE start=True bits
                    # from the emb matmuls stay set, so later matmuls still
                    # accumulate over the DVE-written values)
                    nc.vector.tensor_add(g0_if[:, :], g0_if[:, :], sc_if[:])
                    nc.vector.tensor_add(g0_jo[:, :], g0_jo[:, :], sc_jo[:])
                    for k in range(4):
                        lhs = _hT(h0T, k)
                        last = k == 3
                        for gt, gtile in ((0, g0_if), (512, g0_jo)):
                            for h2 in (0, 1):
                                nc.tensor.matmul(
                                    gtile[64 * h2:64 * h2 + 64, :], lhs,
                                    w0h_sb[k][:, 1024 * h2 + gt:1024 * h2 + gt + 512],
                                    start=False, stop=last)

                    # deferred tail of the previous step: h1 transpose + proj.
                    # Emitted after this step's L0 matmuls so they outrank it
                    # in PE priority (the transpose waits on the slow cell1
                    # chain; L0 work must not queue behind it).
                    if pending is not None:
                        h1T = emit_tail(*pending)
                        pending = None

                    # ----- layer1 h1-part (ready at step start) -----
                    g1_if = pg.tile([128, 512], F32, name="g1if", tag="g1if", bufs=2)
                    g1_jo = pg.tile([128, 512], F32, name="g1jo", tag="g1jo", bufs=2)
                    for k in range(4):
                        lhs = _hT(h1T, k)
                        for gt, gtile in ((0, g1_if), (512, g1_jo)):
                            for h2 in (0, 1):
                                nc.tensor.matmul(
                                    gtile[64 * h2:64 * h2 + 64, :], lhs,
                                    w1_sb[4 + k][:, 1024 * h2 + gt:1024 * h2 + gt + 512],
                                    start=(k == 0), stop=False)
                    if with_b1:
                        for h2 in (0, 1):
                            cb = 1024 * h2
                            ob = 64 * h2
                            nc.tensor.matmul(g1_if[ob:ob + 64, :], ones1_sb[:],
                                             b1_sb[:, cb:cb + 512],
                                             start=False, stop=False)
                            nc.tensor.matmul(g1_jo[ob:ob + 64, :], ones1_sb[:],
                                             b1_sb[:, cb + 512:cb + 1024],
                                             start=False, stop=False)

                    # ----- layer0 cell -----
                    sif0 = pa.tile([128, 512], F32, name="sif0", tag="sif0")
                    nc.scalar.activation(sif0[:], g0_if[:], AF.Sigmoid)
                    tj0 = pa.tile([128, 256], F32, name="tj0", tag="tj0")
                    nc.scalar.activation(tj0[:], g0_jo[:, 0:256], AF.Tanh)
                    so0 = pa.tile([128, 256], F32, name="so0", tag="so0")
                    nc.scalar.activation(so0[:], g0_jo[:, 256:512], AF.Sigmoid)
                    u0 = pc.tile([128, 256], F32, name="u0", tag="u0")
                    nc.vector.tensor_mul(u0[:], c0[:], sif0[:, 256:512])
                    u1 = pc.tile([128, 256], F32, name="u1", tag="u1")
                    nc.gpsimd.tensor_mul(u1[:], sif0[:, 0:256], tj0[:])
                    nc.vector.tensor_add(c0[:], u0[:], u1[:])
                    tc0 = pa.tile([128, 256], F32, name="tc0", tag="tc0")
                    nc.scalar.activation(tc0[:], c0[:], AF.Tanh)
                    h0n = pc.tile([128, 256], BF16, name="h0n", tag="h0n")
                    nc.vector.tensor_mul(h0n[:], tc0[:], so0[:])
                    tp0 = ptp.tile([128, 256], BF16, name="tp0", tag="tp")
                    for blk in (0, 1):
                        sl = slice(128 * blk, 128 * blk + 128)
                        nc.tensor.transpose(tp0[:, sl], h0n[:, sl], identb[:])
                    h0T_new = ph.tile([128, 256], BF16, name="h0T", tag="h0T")
                    nc.vector.tensor_copy(h0T_new[:], tp0[:])
                    h0T = h0T_new

                    # ----- layer1 h0-part -----
                    for k in range(4):
                        lhs = _hT(h0T, k)
                        last = k == 3
                        for gt, gtile in ((0, g1_if), (512, g1_jo)):
                            for h2 in (0, 1):
                                nc.tensor.matmul(
                                    gtile[64 * h2:64 * h2 + 64, :], lhs,
                                    w1_sb[k][:, 1024 * h2 + gt:1024 * h2 + gt + 512],
                                    start=False, stop=last)

                    # ----- layer1 cell (mask clamps via ACT bias) -----
                    si1 = pa.tile([128, 256], F32, name="si1", tag="si1")
                    nc.scalar.activation(si1[:], g1_if[:, 0:256], AF.Sigmoid,
                                         bias=iclamp[:, t:t + 1])
                    sf1 = pa.tile([128, 256], F32, name="sf1", tag="sf1")
                    nc.scalar.activation(sf1[:], g1_if[:, 256:512], AF.Sigmoid,
                                         bias=fclamp[:, t:t + 1])
                    tj1 = pa.tile([128, 256], F32, name="tj1", tag="tj1")
                    nc.scalar.activation(tj1[:], g1_jo[:, 0:256], AF.Tanh)
                    so1 = pa.tile([128, 256], F32, name="so1", tag="so1")
                    nc.scalar.activation(so1[:], g1_jo[:, 256:512], AF.Sigmoid)
                    v0 = pc.tile([128, 256], F32, name="v0", tag="v0")
                    nc.vector.tensor_mul(v0[:], c1[:], sf1[:])
                    v1 = pc.tile([128, 256], F32, name="v1", tag="v1")
                    nc.gpsimd.tensor_mul(v1[:], si1[:], tj1[:])
                    nc.vector.tensor_add(c1[:], v0[:], v1[:])
                    tc1 = pa.tile([128, 256], F32, name="tc1", tag="tc1")
                    nc.scalar.activation(tc1[:], c1[:], AF.Tanh)
                    h1n = pc.tile([128, 256], BF16, name="h1n", tag="h1n")
                    nc.vector.tensor_mul(h1n[:], tc1[:], so1[:])
                    pending = (t, h1n, tc1, so1)

                emit_tail(*pending)

    _split_waits(nc)
    return nc


def _split_waits(nc, max_waits=1):
    # this walrus build rejects instructions carrying more than one sem
    # wait; hoist extras onto preceding NoOps on the same engine
    for fn in nc.m.functions:
        for bb in fn.blocks:
            new_insts = []
            for ins in bb.instructions:
                w = ins.sync_info.on_wait if ins.sync_info else None
                if w and len(w) > max_waits:
                    extra, keep = w[:-max_waits], w[-max_waits:]
                    for i, sw in enumerate(extra):
                        new_insts.append(mybir.InstNoOp(
                            name=f"{ins.name}-wsplit{i}", engine=ins.engine,
                            ins=[], outs=[],
                            sync_info=mybir.SyncInfo(on_wait=[sw], on_update=[])))
                    ins.sync_info = mybir.SyncInfo(
                        on_wait=keep, on_update=ins.sync_info.on_update)
                new_insts.append(ins)
            bb.instructions[:] = new_insts


_module_cache = {}


def _get_module(with_b1, with_bp):
    key = (with_b1, with_bp)
    if key not in _module_cache:
        _module_cache[key] = _build_module(with_b1, with_bp)
    return _module_cache[key]


def kernel(dec_input_batch, dec_input_lengths, input_state_vectors,
           char_embeddings, W0, b0, W1, b1, Wp, bp):
    dec_input_batch = np.asarray(dec_input_batch)
    dec_input_lengths = np.asarray(dec_input_lengths)
    input_state_vectors = np.asarray(input_state_vectors, np.float32)
    char_embeddings = np.asarray(char_embeddings, np.float32)
    W0 = np.asarray(W0, np.float32)
    b0 = np.asarray(b0, np.float32)
    W1 = np.asarray(W1, np.float32)
    b1 = np.asarray(b1, np.float32)
    Wp = np.asarray(Wp, np.float32)
    bp = np.asarray(bp, np.float32)

    perm = _perm_cols()
    w0p = W0[:, perm]
    w1p = W1[:, perm]
    b0p = b0[perm]
    fpat = np.zeros(G, np.float32)
    for g in (0, 1):
        fpat[1024 * g + 256:1024 * g + 512] = 1.0  # f-gate cols, permuted layout
    sign_ce = np.zeros(G, np.float32)
    for g in (0, 1):
        sign_ce[1024 * g:1024 * g + 256] = -CLAMP
        sign_ce[1024 * g + 256:1024 * g + 512] = CLAMP

    with_b1 = bool(np.any(b1))
    with_bp = bool(np.any(bp))
    nc = _get_module(with_b1, with_bp)

    shared = {
        "iota_v": np.arange(V, dtype=np.float32).reshape(V, 1),
        "iota_t": np.arange(128, dtype=np.float32).reshape(128, 1),
        "iota_bc": np.broadcast_to(
            np.arange(128, dtype=np.float32), (128, 128)).copy(),
        "sign_ce": sign_ce.reshape(1, G).astype(ml_dtypes.bfloat16),
        "ones2": np.ones((2, BL), np.float32),
        "et": np.ascontiguousarray(char_embeddings.T),
        "w0e": np.ascontiguousarray(w0p[0:E]),
        "w0s": np.ascontiguousarray(w0p[E:E + H]),
        "w0h": np.ascontiguousarray(w0p[E + H:]).astype(ml_dtypes.bfloat16),
        "w1": np.ascontiguousarray(w1p).astype(ml_dtypes.bfloat16),
        "wp": Wp,
        "b0f": np.ascontiguousarray(np.stack([b0p, fpat])),
    }
    if with_b1:
        shared["b1p"] = np.ascontiguousarray(b1[perm].reshape(1, G))
    if with_b1 or with_bp:
        shared["ones1"] = np.ones((1, BL), np.float32)
    if with_bp:
        shared["bp"] = bp.reshape(1, V)

    in_maps = []
    for c in range(NCORES):
        sl = slice(BL * c, BL * (c + 1))
        tok = dec_input_batch[sl].astype(np.float32)          # [BL, T]
        tok_row = np.ascontiguousarray(tok.T).reshape(1, T * BL)
        lens = dec_input_lengths[sl].astype(np.float32)
        m = dict(shared)
        m["tok_bc"] = np.broadcast_to(tok_row, (V, T * BL)).copy()
        m["len_bc"] = np.broadcast_to(lens, (128, BL)).copy()
        m["len_col"] = np.tile(lens, 2).reshape(128, 1).copy()
        m["stateT"] = np.ascontiguousarray(input_state_vectors[sl].T)
        in_maps.append(m)

    global _last_in_maps
    _last_in_maps = in_maps
    res = run_bass_kernel_spmd(nc, in_maps, list(range(NCORES)))
    out = np.concatenate(
        [r["scores"].transpose(1, 0, 2) for r in res.results], axis=0)
    return np.ascontiguousarray(out, dtype=np.float32)


_last_in_maps = None


if __name__ == "__main__":
    pass


# revision 30
# speedup vs baseline: 1.1593x; 1.1593x over previous
"""Trainium2 Bass kernel: 2-layer LSTM decoder (B=512 T=128 E=64 H=512 V=100).

Strategy: pure data parallelism, batch 512 -> 8 cores x 64.
Per core, everything is fp32 and all weights stay resident in SBUF.

Matmul mapping ("x-stationary"): out = gates[batch, feат] in PSUM,
stationary lhsT = transposed activations [K,64], moving rhs = weights
[K, cols]. The 128-wide PE array is split into two 64-col groups
(tile_position) so both batch-halves of the gate features compute
concurrently: PSUM gate tiles are [128, 512] = [batch(64) x 2 feature
halves, i|f or j|o gate pair], host pre-permutes weight columns into
[i_lo f_lo j_lo o_lo | i_hi f_hi j_hi o_hi] blocks of 256.

Cell elementwise runs on [128, 256] tiles (feature halves stacked on
partitions). dynamic_rnn length masking is exact and branch-free:
  - layer0: a rider row on the embedding one-hot matmul adds
    -1e9*(t>=len) to i-gate pre-activations and +1e9*(t>=len) to f-gate
    (c then freezes exactly; h past the length is garbage but provably
    never reaches the output).
  - layer1: same clamp via per-partition ACT bias columns.
  - output: scores_t = (mask * h1_t) @ Wp [+ bp].
Embedding lookup = one-hot matmul against a precomputed E @ W0_emb
table; the state (t-invariant) contribution + b0 + forget_bias is
precomputed once and re-added to PSUM each step via an identity matmul.
"""

import sys
import types

import ml_dtypes
import numpy as np

import concourse.bass as bass
import concourse.mybir as mybir
import concourse.tile as tile
from concourse.bass_utils import run_bass_kernel_spmd
from concourse.masks import make_identity

B, T, E, H, V = 512, 128, 64, 512, 100
NCORES = 8
BL = B // NCORES          # 64 rows of batch per core
G = 4 * H                 # 2048 gate columns
F32 = mybir.dt.float32
F32R = mybir.dt.float32r
BF16 = mybir.dt.bfloat16
AF = mybir.ActivationFunctionType
ALU = mybir.AluOpType
CLAMP = 1.0e9


def _perm_cols():
    # reference gate order is [i j f o]*512; regroup to
    # [i_lo f_lo j_lo o_lo | i_hi f_hi j_hi o_hi] blocks of 256
    idx = []
    for g in (0, 1):
        lo = 256 * g
        for base in (0, 1024, 512, 1536):  # i, f, j, o
            idx.extend(range(base + lo, base + lo + 256))
    return np.asarray(idx)


def _hT(tile_, k):
    # transposed-activation K-chunk k (feats 128k:128k+128) as [128, 64];
    # tile_ is [128, 256] = [block0 (feats 0:128,256:384) | block1]
    off = 128 * (k % 2) + 64 * (k // 2)
    return tile_[:, off:off + 64]


def _build_module(with_b1, with_bp):
    nc = bass.Bass(target_bir_lowering=False)

    def din(name, shape, dt=F32):
        return nc.dram_tensor(name, list(shape), dt, kind="ExternalInput")

    tok_bc = din("tok_bc", (V, T * BL))       # token id per (t,b) col, replicated
    len_bc = din("len_bc", (128, BL))         # lengths replicated over partitions
    len_col = din("len_col", (128, 1))        # lengths duplicated per partition half
    iota_v = din("iota_v", (V, 1))            # 0..99
    iota_t = din("iota_t", (128, 1))          # 0..127
    iota_bc = din("iota_bc", (128, 128))      # 0..127 per col, replicated
    sign_ce = din("sign_ce", (1, G), BF16)          # -+1e9 on i/f cols, 0 on j/o
    ones2 = din("ones2", (2, BL), F32R)
    stateT = din("stateT", (H, BL), F32R)
    et = din("et", (E, V), F32R)                    # char_embeddings.T
    w0e = din("w0e", (E, G), F32R)                  # permuted column blocks
    w0s = din("w0s", (H, G), F32R)
    w0h = din("w0h", (H, G), BF16)
    w1 = din("w1", (2 * H, G), BF16)
    wp = din("wp", (H, V), F32R)
    b0f = din("b0f", (2, G), F32R)                  # [perm(b0); +1 on f cols]
    b1p = din("b1p", (1, G), F32R) if with_b1 else None
    ones1 = din("ones1", (1, BL), F32R) if (with_bp or with_b1) else None
    bpp = din("bp", (1, V), F32R) if with_bp else None

    scores = nc.dram_tensor("scores", [T, BL, V], F32, kind="ExternalOutput")

    with tile.TileContext(nc) as tc:
        with tc.tile_pool(name="persist", bufs=1) as pp:
            # ---------------- persistent SBUF ----------------
            w0h_sb = [pp.tile([128, G], BF16, name=f"w0h{k}", tag=f"w0h{k}") for k in range(4)]
            w1_sb = [pp.tile([128, G], BF16, name=f"w1{k}", tag=f"w1{k}") for k in range(8)]
            wp_sb = [pp.tile([128, V], F32R, name=f"wp{k}", tag=f"wp{k}") for k in range(4)]
            wp16_sb = [pp.tile([128, V], BF16, name=f"wpb{k}", tag=f"wpb{k}") for k in range(4)]
            oh = pp.tile([V + 1, T * BL], BF16, name="oh", tag="oh")
            ce = pp.tile([V + 1, G], BF16, name="ce", tag="ce")
            sc_if = pp.tile([128, 512], F32R, name="sc_if", tag="sc_if")
            sc_jo = pp.tile([128, 512], F32R, name="sc_jo", tag="sc_jo")
            identf = pp.tile([128, 128], F32, name="identf", tag="identf")
            ident = pp.tile([128, 128], F32R, name="ident", tag="ident")
            identb = pp.tile([128, 128], BF16, name="identb", tag="identb")
            maskcols = pp.tile([128, 128], F32, name="maskcols", tag="maskcols")
            iclamp = pp.tile([128, 128], F32, name="iclamp", tag="iclamp")
            fclamp = pp.tile([128, 128], F32, name="fclamp", tag="fclamp")
            invm = pp.tile([128, BL], BF16, name="invm", tag="invm")
            ones2_sb = pp.tile([2, BL], F32R, name="ones2", tag="ones2")
            c0 = pp.tile([128, 256], F32, name="c0", tag="c0")
            c1 = pp.tile([128, 256], F32, name="c1", tag="c1")
            hz = [pp.tile([128, 256], BF16, name=f"hz{i}", tag=f"hz{i}") for i in range(2)]
            if with_b1:
                b1_sb = pp.tile([1, G], F32R, name="b1", tag="b1")
            if with_b1 or with_bp:
                ones1_sb = pp.tile([1, BL], F32R, name="ones1", tag="ones1")
            if with_bp:
                bp_sb = pp.tile([1, V], F32R, name="bp", tag="bp")

            for k in range(4):
                nc.sync.dma_start(w0h_sb[k][:], w0h[128 * k:128 * k + 128, :])
            for k in range(4):
                nc.sync.dma_start(w1_sb[k][:], w1[128 * k:128 * k + 128, :])
                nc.sync.dma_start(w1_sb[4 + k][:], w1[512 + 128 * k:512 + 128 * k + 128, :])
            for k in range(4):
                nc.sync.dma_start(wp_sb[k][:], wp[128 * k:128 * k + 128, :])
                nc.vector.tensor_copy(wp16_sb[k][:], wp_sb[k][:])
            nc.sync.dma_start(ones2_sb[:], ones2[:])
            if with_b1:
                nc.sync.dma_start(b1_sb[:], b1p[:])
            if with_b1 or with_bp:
                nc.sync.dma_start(ones1_sb[:], ones1[:])
            if with_bp:
                nc.sync.dma_start(bp_sb[:], bpp[:])

            make_identity(nc, identf[:])
            nc.vector.tensor_copy(ident[:], identf[:])
            nc.vector.tensor_copy(identb[:], identf[:])
            nc.gpsimd.memset(c0[:], 0.0)
            nc.gpsimd.memset(c1[:], 0.0)
            zf = pp.tile([128, 256], F32, name="zf", tag="zf")
            nc.gpsimd.memset(zf[:], 0.0)
            for z in hz:
                nc.vector.tensor_copy(z[:], zf[:])

            # ---------------- startup compute ----------------
            with (
                tc.tile_pool(name="boot", bufs=2) as bp_pool,
                tc.tile_pool(name="boot1", bufs=1) as b1_pool,
                tc.tile_pool(name="psum_boot", bufs=1, space="PSUM") as pb,
            ):
                # masks
                lb = b1_pool.tile([128, BL], F32, name="len_bc", tag="len_bc")
                nc.sync.dma_start(lb[:], len_bc[:])
                lc = b1_pool.tile([128, 1], F32, name="len_col", tag="len_col")
                nc.sync.dma_start(lc[:], len_col[:])
                it_ = b1_pool.tile([128, 1], F32, name="iota_t", tag="iota_t")
                nc.sync.dma_start(it_[:], iota_t[:])
                ib = b1_pool.tile([128, 128], F32, name="iota_bc", tag="iota_bc")
                nc.sync.dma_start(ib[:], iota_bc[:])
                iv = b1_pool.tile([V, 1], F32, name="iota_v", tag="iota_v")
                nc.sync.dma_start(iv[:], iota_v[:])

                # invm[t,b] = (len[b] <= t)
                nc.vector.tensor_scalar(invm[:], lb[:], it_[:], None, ALU.is_le)
                # maskcols[p,t] = (t < len[p%64])
                nc.vector.tensor_scalar(maskcols[:], ib[:], lc[:], None, ALU.is_lt)
                # iclamp = (m-1)*1e9 ; fclamp = (1-m)*1e9 + 1
                nc.vector.tensor_scalar(iclamp[:], maskcols[:], 1.0, CLAMP,
                                        ALU.subtract, ALU.mult)
                nc.vector.tensor_scalar(fclamp[:], maskcols[:], 1.0, -CLAMP,
                                        ALU.subtract, ALU.mult)
                nc.vector.tensor_scalar_add(fclamp[:], fclamp[:], 1.0)

                # one-hot token table, built in column chunks
                CH = 1024
                for cidx in range(T * BL // CH):
                    tb = bp_pool.tile([V, CH], F32, name="tok_bc", tag="tok_bc")
                    nc.sync.dma_start(tb[:], tok_bc[:, CH * cidx:CH * (cidx + 1)])
                    nc.vector.tensor_scalar(oh[0:V, CH * cidx:CH * (cidx + 1)],
                                            tb[:], iv[:], None, ALU.is_equal)
                # rider row: invm flattened t-major
                nc.sync.dma_start(oh[V:V + 1, :], invm[:])

                # ce = [E @ w0e ; sign row]
                et_sb = b1_pool.tile([E, V], F32R, name="et", tag="et")
                nc.sync.dma_start(et_sb[:], et[:])
                w0e_sb = b1_pool.tile([E, G], F32R, name="w0e", tag="w0e")
                nc.sync.dma_start(w0e_sb[:], w0e[:])
                for n in range(4):
                    cps = pb.tile([V, 512], F32, name="ce_ps", tag="ce_ps")
                    nc.tensor.matmul(cps[:], et_sb[:], w0e_sb[:, 512 * n:512 * n + 512],
                                     start=True, stop=True)
                    nc.scalar.copy(ce[0:V, 512 * n:512 * n + 512], cps[:])
                nc.sync.dma_start(ce[V:V + 1, :], sign_ce[:])

                # sc = state @ w0s + b0 + forget_bias, in gate-tile layout
                st_sb = [b1_pool.tile([128, BL], F32R, name=f"st{k}", tag=f"st{k}") for k in range(4)]
                for k in range(4):
                    nc.sync.dma_start(st_sb[k][:], stateT[128 * k:128 * k + 128, :])
                b0f_sb = b1_pool.tile([2, G], F32R, name="b0f", tag="b0f")
                nc.sync.dma_start(b0f_sb[:], b0f[:])
                for h2 in (0, 1):
                    cb = 1024 * h2
                    ob = 64 * h2
                    ps_if = pb.tile([64, 512], F32, name="sc_ps_if", tag="sc_ps_if")
                    ps_jo = pb.tile([64, 512], F32, name="sc_ps_jo", tag="sc_ps_jo")
                    for k in range(4):
                        w0s_k = bp_pool.tile([128, G], F32R, name="w0s", tag="w0s")
                        nc.sync.dma_start(w0s_k[:], w0s[128 * k:128 * k + 128, :])
                        nc.tensor.matmul(ps_if[:], st_sb[k][:], w0s_k[:, cb:cb + 512],
                                         start=(k == 0), stop=False)
                        nc.tensor.matmul(ps_jo[:], st_sb[k][:],
                                         w0s_k[:, cb + 512:cb + 1024],
                                         start=(k == 0), stop=False)
                    nc.tensor.matmul(ps_if[:], ones2_sb[:], b0f_sb[:, cb:cb + 512],
                                     start=False, stop=True)
                    nc.tensor.matmul(ps_jo[:], ones2_sb[:],
                                     b0f_sb[:, cb + 512:cb + 1024],
                                     start=False, stop=True)
                    nc.scalar.copy(sc_if[64 * h2:64 * h2 + 64, :], ps_if[:])
                    nc.scalar.copy(sc_jo[64 * h2:64 * h2 + 64, :], ps_jo[:])

            # ---------------- recurrence ----------------
            with (
                tc.tile_pool(name="act", bufs=4) as pa,
                tc.tile_pool(name="cell", bufs=4) as pc,
                tc.tile_pool(name="ht", bufs=4) as ph,
                tc.tile_pool(name="psum_g", bufs=1, space="PSUM") as pg,
                tc.tile_pool(name="psum_tp", bufs=1, space="PSUM") as ptp,
                tc.tile_pool(name="psum_s", bufs=1, space="PSUM") as psp,
            ):
                h0T = hz[0]
                h1T = hz[1]
                pending = None  # (t, h1n[, tc1, so1]) deferred tail

                def emit_tail(pt, h1n_p, tc1_p, so1_p):
                    # h1 transpose for the recurrence
                    tp1 = ptp.tile([128, 256], BF16, name="tp1", tag="tp")
                    for blk in (0, 1):
                        sl = slice(128 * blk, 128 * blk + 128)
                        nc.tensor.transpose(tp1[:, sl], h1n_p[:, sl], identb[:])
                    hT = ph.tile([128, 256], BF16, name="h1T", tag="h1T")
                    nc.vector.tensor_copy(hT[:], tp1[:])
                    # masked projection
                    s = psp.tile([BL, V], F32, name="s", tag="s")
                    if with_bp:
                        h1m = pc.tile([128, 256], F32R, name="h1m", tag="h1m")
                        nc.vector.scalar_tensor_tensor(
                            h1m[:], tc1_p[:], maskcols[:, pt:pt + 1], so1_p[:],
                            ALU.mult, ALU.mult)
                        tpm = ptp.tile([128, 256], F32R, name="tpm", tag="tpr",
                                       bufs=1)
                        for blk in (0, 1):
                            nc.tensor.transpose(
                                tpm[:, 128 * blk:128 * blk + 128],
                                h1m[:, 128 * blk:128 * blk + 128], ident[:])
                        hmT = ph.tile([128, 256], F32R, name="hmT", tag="hmT")
                        nc.vector.tensor_copy(hmT[:], tpm[:])
                        for k in range(4):
                            nc.tensor.matmul(s[:], _hT(hmT, k), wp_sb[k][:],
                                             start=(k == 0), stop=False)
                        nc.tensor.matmul(s[:], ones1_sb[:], bp_sb[:],
                                         start=False, stop=True)
                        s_sb = pc.tile([BL, V], F32, name="s_sb", tag="s_sb")
                        nc.scalar.copy(s_sb[:], s[:])
                    else:
                        for k in range(4):
                            nc.tensor.matmul(s[:], _hT(hT, k), wp16_sb[k][:],
                                             start=(k == 0), stop=(k == 3))
                        s_sb = pc.tile([BL, V], F32, name="s_sb", tag="s_sb")
                        nc.vector.tensor_scalar_mul(s_sb[:], s[:],
                                                    maskcols[0:BL, pt:pt + 1])
                    nc.sync.dma_start(scores[pt], s_sb[:])
                    return hT

                for t in range(T):
                    ohs = oh[:, BL * t:BL * (t + 1)]

                    # ----- layer0 gate accumulation -----
                    g0_if = pg.tile([128, 512], F32, name="g0if", tag="g0if")
                    g0_jo = pg.tile([128, 512], F32, name="g0jo", tag="g0jo")
                    for gt, gtile in ((0, g0_if), (512, g0_jo)):
                        for h2 in (0, 1):
                            nc.tensor.matmul(
                                gtile[64 * h2:64 * h2 + 64, :], ohs,
                                ce[:, 1024 * h2 + gt:1024 * h2 + gt + 512],
                                start=True, stop=False)
                    # state contribution added on DVE, early (before the
                    # h-part matmuls accumulate on top; PE start=True bits
                    # from the emb matmuls stay set, so later matmuls still
                    # accumulate over the DVE-written values)
                    nc.vector.tensor_add(g0_if[:, :], g0_if[:, :], sc_if[:])
                    nc.vector.tensor_add(g0_jo[:, :], g0_jo[:, :], sc_jo[:])
                    for gt, gtile in ((0, g0_if), (512, g0_jo)):
                        for k in range(4):
                            lhs = _hT(h0T, k)
                            last = k == 3
                            for h2 in (0, 1):
                                nc.tensor.matmul(
                                    gtile[64 * h2:64 * h2 + 64, :], lhs,
                                    w0h_sb[k][:, 1024 * h2 + gt:1024 * h2 + gt + 512],
                                    start=False, stop=last)

                    # deferred tail of the previous step: h1 transpose + proj.
                    # Emitted after this step's L0 matmuls so they outrank it
                    # in PE priority (the transpose waits on the slow cell1
                    # chain; L0 work must not queue behind it).
                    if pending is not None:
                        h1T = emit_tail(*pending)
                        pending = None

                    # ----- layer1 h1-part (ready at step start) -----
                    g1_if = pg.tile([128, 512], F32, name="g1if", tag="g1if", bufs=2)
                    g1_jo = pg.tile([128, 512], F32, name="g1jo", tag="g1jo", bufs=2)
                    for k in range(4):
                        lhs = _hT(h1T, k)
                        for gt, gtile in ((0, g1_if), (512, g1_jo)):
                            for h2 in (0, 1):
                                nc.tensor.matmul(
                                    gtile[64 * h2:64 * h2 + 64, :], lhs,
                                    w1_sb[4 + k][:, 1024 * h2 + gt:1024 * h2 + gt + 512],
                                    start=(k == 0), stop=False)
                    if with_b1:
                        for h2 in (0, 1):
                            cb = 1024 * h2
                            ob = 64 * h2
                            nc.tensor.matmul(g1_if[ob:ob + 64, :], ones1_sb[:],
                                             b1_sb[:, cb:cb + 512],
                                             start=False, stop=False)
                            nc.tensor.matmul(g1_jo[ob:ob + 64, :], ones1_sb[:],
                                             b1_sb[:, cb + 512:cb + 1024],
                                             start=False, stop=False)

                    # ----- layer0 cell -----
                    sif0 = pa.tile([128, 512], F32, name="sif0", tag="sif0")
                    nc.scalar.activation(sif0[:], g0_if[:], AF.Sigmoid)
                    tj0 = pa.tile([128, 256], F32, name="tj0", tag="tj0")
                    nc.scalar.activation(tj0[:], g0_jo[:, 0:256], AF.Tanh)
                    so0 = pa.tile([128, 256], F32, name="so0", tag="so0")
                    nc.scalar.activation(so0[:], g0_jo[:, 256:512], AF.Sigmoid)
                    u0 = pc.tile([128, 256], F32, name="u0", tag="u0")
                    nc.vector.tensor_mul(u0[:], c0[:], sif0[:, 256:512])
                    u1 = pc.tile([128, 256], F32, name="u1", tag="u1")
                    nc.vector.tensor_mul(u1[:], sif0[:, 0:256], tj0[:])
                    nc.vector.tensor_add(c0[:], u0[:], u1[:])
                    tc0 = pa.tile([128, 256], F32, name="tc0", tag="tc0")
                    nc.scalar.activation(tc0[:], c0[:], AF.Tanh)
                    h0n = pc.tile([128, 256], BF16, name="h0n", tag="h0n")
                    nc.vector.tensor_mul(h0n[:], tc0[:], so0[:])
                    tp0 = ptp.tile([128, 256], BF16, name="tp0", tag="tp")
                    for blk in (0, 1):
                        sl = slice(128 * blk, 128 * blk + 128)
                        nc.tensor.transpose(tp0[:, sl], h0n[:, sl], identb[:])
                    h0T_new = ph.tile([128, 256], BF16, name="h0T", tag="h0T")
                    nc.vector.tensor_copy(h0T_new[:], tp0[:])
                    h0T = h0T_new

                    # ----- layer1 h0-part -----
                    for gt, gtile in ((0, g1_if), (512, g1_jo)):
                        for k in range(4):
                            lhs = _hT(h0T, k)
                            last = k == 3
                            for h2 in (0, 1):
                                nc.tensor.matmul(
                                    gtile[64 * h2:64 * h2 + 64, :], lhs,
                                    w1_sb[k][:, 1024 * h2 + gt:1024 * h2 + gt + 512],
                                    start=False, stop=last)

                    # ----- layer1 cell (mask clamps via ACT bias) -----
                    si1 = pa.tile([128, 256], F32, name="si1", tag="si1")
                    nc.scalar.activation(si1[:], g1_if[:, 0:256], AF.Sigmoid,
                                         bias=iclamp[:, t:t + 1])
                    sf1 = pa.tile([128, 256], F32, name="sf1", tag="sf1")
                    nc.scalar.activation(sf1[:], g1_if[:, 256:512], AF.Sigmoid,
                                         bias=fclamp[:, t:t + 1])
                    tj1 = pa.tile([128, 256], F32, name="tj1", tag="tj1")
                    nc.scalar.activation(tj1[:], g1_jo[:, 0:256], AF.Tanh)
                    so1 = pa.tile([128, 256], F32, name="so1", tag="so1")
                    nc.scalar.activation(so1[:], g1_jo[:, 256:512], AF.Sigmoid)
                    v0 = pc.tile([128, 256], F32, name="v0", tag="v0")
                    nc.vector.tensor_mul(v0[:], c1[:], sf1[:])
                    v1 = pc.tile([128, 256], F32, name="v1", tag="v1")
                    nc.vector.tensor_mul(v1[:], si1[:], tj1[:])
                    nc.vector.tensor_add(c1[:], v0[:], v1[:])
                    tc1 = pa.tile([128, 256], F32, name="tc1", tag="tc1")
                    nc.scalar.activation(tc1[:], c1[:], AF.Tanh)
                    h1n = pc.tile([128, 256], BF16, name="h1n", tag="h1n")
                    nc.vector.tensor_mul(h1n[:], tc1[:], so1[:])
                    pending = (t, h1n, tc1, so1)

                emit_tail(*pending)

    _split_waits(nc)
    return nc


def _split_waits(nc, max_waits=1):
    # this walrus build rejects instructions carrying more than one sem
    # wait; hoist extras onto preceding NoOps on the same engine
    for fn in nc.m.functions:
        for bb in fn.blocks:
            new_insts = []
            for ins in bb.instructions:
                w = ins.sync_info.on_wait if ins.sync_info else None
                if w and len(w) > max_waits:
                    extra, keep = w[:-max_waits], w[-max_waits:]
                    for i, sw in enumerate(extra):
                        new_insts.append(mybir.InstNoOp(
                            name=f"{ins.name}-wsplit{i}", engine=ins.engine,
                            ins=[], outs=[],
                            sync_info=mybir.SyncInfo(on_wait=[sw], on_update=[])))
                    ins.sync_info = mybir.SyncInfo(
                        on_wait=keep, on_update=ins.sync_info.on_update)
                new_insts.append(ins)
            bb.instructions[:] = new_insts


_module_cache = {}


def _get_module(with_b1, with_bp):
    key = (with_b1, with_bp)
    if key not in _module_cache:
        _module_cache[key] = _build_module(with_b1, with_bp)
    return _module_cache[key]


def kernel(dec_input_batch, dec_input_lengths, input_state_vectors,
           char_embeddings, W0, b0, W1, b1, Wp, bp):
    dec_input_batch = np.asarray(dec_input_batch)
    dec_input_lengths = np.asarray(dec_input_lengths)
    input_state_vectors = np.asarray(input_state_vectors, np.float32)
    char_embeddings = np.asarray(char_embeddings, np.float32)
    W0 = np.asarray(W0, np.float32)
    b0 = np.asarray(b0, np.float32)
    W1 = np.asarray(W1, np.float32)
    b1 = np.asarray(b1, np.float32)
    Wp = np.asarray(Wp, np.float32)
    bp = np.asarray(bp, np.float32)

    perm = _perm_cols()
    w0p = W0[:, perm]
    w1p = W1[:, perm]
    b0p = b0[perm]
    fpat = np.zeros(G, np.float32)
    for g in (0, 1):
        fpat[1024 * g + 256:1024 * g + 512] = 1.0  # f-gate cols, permuted layout
    sign_ce = np.zeros(G, np.float32)
    for g in (0, 1):
        sign_ce[1024 * g:1024 * g + 256] = -CLAMP
        sign_ce[1024 * g + 256:1024 * g + 512] = CLAMP

    with_b1 = bool(np.any(b1))
    with_bp = bool(np.any(bp))
    nc = _get_module(with_b1, with_bp)

    shared = {
        "iota_v": np.arange(V, dtype=np.float32).reshape(V, 1),
        "iota_t": np.arange(128, dtype=np.float32).reshape(128, 1),
        "iota_bc": np.broadcast_to(
            np.arange(128, dtype=np.float32), (128, 128)).copy(),
        "sign_ce": sign_ce.reshape(1, G).astype(ml_dtypes.bfloat16),
        "ones2": np.ones((2, BL), np.float32),
        "et": np.ascontiguousarray(char_embeddings.T),
        "w0e": np.ascontiguousarray(w0p[0:E]),
        "w0s": np.ascontiguousarray(w0p[E:E + H]),
        "w0h": np.ascontiguousarray(w0p[E + H:]).astype(ml_dtypes.bfloat16),
        "w1": np.ascontiguousarray(w1p).astype(ml_dtypes.bfloat16),
        "wp": Wp,
        "b0f": np.ascontiguousarray(np.stack([b0p, fpat])),
    }
    if with_b1:
        shared["b1p"] = np.ascontiguousarray(b1[perm].reshape(1, G))
    if with_b1 or with_bp:
        shared["ones1"] = np.ones((1, BL), np.float32)
    if with_bp:
        shared["bp"] = bp.reshape(1, V)

    in_maps = []
    for c in range(NCORES):
        sl = slice(BL * c, BL * (c + 1))
        tok = dec_input_batch[sl].astype(np.float32)          # [BL, T]
        tok_row = np.ascontiguousarray(tok.T).reshape(1, T * BL)
        lens = dec_input_lengths[sl].astype(np.float32)
        m = dict(shared)
        m["tok_bc"] = np.broadcast_to(tok_row, (V, T * BL)).copy()
        m["len_bc"] = np.broadcast_to(lens, (128, BL)).copy()
        m["len_col"] = np.tile(lens, 2).reshape(128, 1).copy()
        m["stateT"] = np.ascontiguousarray(input_state_vectors[sl].T)
        in_maps.append(m)

    global _last_in_maps
    _last_in_maps = in_maps
    res = run_bass_kernel_spmd(nc, in_maps, list(range(NCORES)))
    out = np.concatenate(
        [r["scores"].transpose(1, 0, 2) for r in res.results], axis=0)
    return np.ascontiguousarray(out, dtype=np.float32)


_last_in_maps = None


if __name__ == "__main__":
    pass
